# revision 2
# baseline (speedup 1.0000x reference)
"""Trainium2 Bass kernel for nn_Ensemble (dense MLP ensemble, E=8, B=65536).

Network (per ensemble member e):
    x   = concat(inputs[..., :48], clip(inputs[..., 48:64], -1, 1))   # [B, 64]
    h1  = relu(x @ W1[e] + b1[e])                                     # [B, 128]
    h2  = relu(h1 @ W2[e] + b2[e])                                    # [B, 128]
    out = h2 @ W3[e] + b3[e]                                          # [B, 48]

Sharding: ensemble dim E=8 across the 8 NeuronCores (one member per core,
weights core-resident).  Feature-major layout: X = [128, B/2] bf16 with the
two batch halves stacked on the partition axis (rows 0:64 / 64:128), so all
three layers are weight-stationary matmuls streaming batch columns.

This problem is bound by PSUM evacuation: matmul output is fp32-only on TRN2
and only the Scalar (1.2 GHz) and Vector (0.96 GHz) engines can read PSUM at
1 elem/cycle/lane, so the h1/h2/out drains (163840 lane-elems) cost ~95us of
engine time split across the two engines; the PE stream (6 matmuls/unit incl
weight swaps, ~1.28us/unit HW-measured) is a close second.  The design keeps
both subsystems busy simultaneously:

  - unit = 512 x-columns: L1 = 2 concurrent row-tiled K=64 matmuls (PE
    row-group concurrency, HW-verified) -> h1ps [128,1024] fp32 (2 banks);
    L2 = 2 dense K=128 matmuls; L3 = 2 concurrent col-tiled M=64 matmuls
    packing 2 batch tiles per bank, output tiles merged pairwise so out
    drains are 1024 wide.
  - software-pipelined emission with skew 2: iteration u emits L1(u),
    L2(u-2), L3(u-4).  Engines execute FIFO, so this ordering lets the PE
    run ahead of the drains and gives every drain a full iteration of sem
    margin (measured ~7us win over skew 1, ~35us over no skew).
  - PSUM: h1/h2 share a 3-slot pool (6 banks; the rotation reproduces only
    the natural d1(u)->L2(u) dependency), merged out tiles use 1x2 banks.
  - each drain op goes to Scalar or Vector by ns-weighted greedy balance
    using HW-measured op costs (ACT ~N/1.2+330ns, DVE ~N/0.96+250ns).
  - x chunks (2MB) prefetched one chunk ahead; the timing variant uses
    For_i(staggered_reset=True) so iterations overlap instead of paying a
    full pipeline drain + all-engine barrier per rep (~20us win).
"""

import numpy as np
import ml_dtypes

BF16 = ml_dtypes.bfloat16

E = 8
B = 65536
HB = B // 2          # batch half (free-dim columns per core)
IN = 64
AC = 16              # clipped action features (last 16)
H = 128
OUT = 48
OUTP = 64            # padded out features (col-group alignment)

CHUNK = 8192         # free-dim columns per x/out DMA chunk
NT = 512             # matmul free dim (one PSUM bank of fp32)
UNIT = 512           # x-columns per pipeline unit
NU = HB // UNIT      # total units (64)
UPC = CHUNK // UNIT  # units per chunk (16)
SKEWD = 2            # software-pipeline skew distance

_CACHED = None


def _build_nc(reps=None):
    """Build the bass module. reps=None -> plain kernel; reps=R wraps the
    body in a hardware For_i loop (self-timing variant)."""
    import contextlib
    import concourse.bacc as bacc
    import concourse.mybir as mybir
    import concourse.tile as tile

    f32 = mybir.dt.float32
    bf16 = mybir.dt.bfloat16
    AF = mybir.ActivationFunctionType
    ALU = mybir.AluOpType

    nc = bacc.Bacc("TRN2", target_bir_lowering=False)

    x_d = nc.dram_tensor("x", [128, HB], bf16, kind="ExternalInput")
    w1_d = nc.dram_tensor("w1p", [128, H], bf16, kind="ExternalInput")
    w2_d = nc.dram_tensor("w2", [H, H], bf16, kind="ExternalInput")
    w3_d = nc.dram_tensor("w3p", [H, OUTP], bf16, kind="ExternalInput")
    b1_d = nc.dram_tensor("b1v", [H, 1], f32, kind="ExternalInput")
    b2_d = nc.dram_tensor("b2v", [H, 1], f32, kind="ExternalInput")
    b3_d = nc.dram_tensor("b3v", [128, 1], f32, kind="ExternalInput")
    out_d = nc.dram_tensor("out", [128, HB], bf16, kind="ExternalOutput")

    with tile.TileContext(nc) as tc:
        with (
            tc.tile_pool(name="consts", bufs=1) as consts,
            tc.tile_pool(name="xp", bufs=2) as xp,
            tc.tile_pool(name="h1sb", bufs=6) as h1pool,
            tc.tile_pool(name="h2sb", bufs=6) as h2pool,
            tc.tile_pool(name="osb", bufs=2) as opool,
            tc.tile_pool(name="psh", bufs=3, space="PSUM") as psh,
            tc.tile_pool(name="pso", bufs=1, space="PSUM") as pso,
        ):
            w1_sb = consts.tile([128, H], bf16)
            w2_sb = consts.tile([H, H], bf16)
            w3_sb = consts.tile([H, OUTP], bf16)
            b1_sb = consts.tile([H, 1], f32)
            b2_sb = consts.tile([H, 1], f32)
            b3_sb = consts.tile([128, 1], f32)
            nc.sync.dma_start(out=w1_sb, in_=w1_d[:])
            nc.sync.dma_start(out=w2_sb, in_=w2_d[:])
            nc.sync.dma_start(out=w3_sb, in_=w3_d[:])
            nc.sync.dma_start(out=b1_sb, in_=b1_d[:])
            nc.sync.dma_start(out=b2_sb, in_=b2_d[:])
            nc.sync.dma_start(out=b3_sb, in_=b3_d[:])

            # ns-weighted greedy balance across the two PSUM-capable engines
            eng_load = {"act": 0.0, "dve": 0.0}

            def drain(dst, src, kind, bias):
                n = dst.shape[-1]
                cost_act = n / 1.2 + 330.0
                cost_dve = n / 0.96 + 250.0
                if eng_load["act"] + cost_act <= eng_load["dve"] + cost_dve:
                    eng_load["act"] += cost_act
                    fn = AF.Relu if kind == "relu" else AF.Identity
                    nc.scalar.activation(dst, src, fn, bias=bias)
                else:
                    eng_load["dve"] += cost_dve
                    if kind == "relu":
                        nc.vector.tensor_scalar(dst, src, bias, 0.0,
                                                op0=ALU.add, op1=ALU.max)
                    else:
                        nc.vector.tensor_scalar_add(dst, src, bias)

            def body():
                x_tiles = {}
                o_tiles = {}
                h1_tiles = {}
                h2_tiles = {}
                ops_pair = {}

                def fetch_x(c):
                    x_t = xp.tile([128, CHUNK], bf16, tag="x", name=f"x_t{c}")
                    nc.sync.dma_start(
                        out=x_t, in_=x_d[:, c * CHUNK:(c + 1) * CHUNK])
                    x_tiles[c] = x_t

                def stage_l1(u):
                    c = u // UPC
                    if u == 0:
                        fetch_x(0)
                    # prefetch next chunk one unit into the current one
                    if u % UPC == 1 and c + 1 < HB // CHUNK:
                        fetch_x(c + 1)
                    x_t = x_tiles[c]
                    h1ps = psh.tile([128, 2 * UNIT], f32, tag="ph",
                                    name=f"h1ps{u}")
                    xo = (u % UPC) * UNIT
                    nc.tensor.matmul(h1ps[:, 0:NT], w1_sb[0:64, :],
                                     x_t[0:64, xo:xo + NT],
                                     start=True, stop=True)
                    nc.tensor.matmul(h1ps[:, NT:2 * NT], w1_sb[64:128, :],
                                     x_t[64:128, xo:xo + NT],
                                     start=True, stop=True)
                    h1sb = h1pool.tile([128, 2 * UNIT], bf16, tag="h1sb",
                                       name=f"h1sb{u}")
                    drain(h1sb, h1ps, "relu", b1_sb)
                    h1_tiles[u] = h1sb

                def stage_l2(u):
                    h1sb = h1_tiles.pop(u)
                    h2ps = psh.tile([128, 2 * UNIT], f32, tag="ph",
                                    name=f"h2ps{u}")
                    nc.tensor.matmul(h2ps[:, 0:NT], w2_sb,
                                     h1sb[:, 0:NT], start=True, stop=True)
                    nc.tensor.matmul(h2ps[:, NT:2 * NT], w2_sb,
                                     h1sb[:, NT:2 * NT], start=True, stop=True)
                    h2sb = h2pool.tile([128, 2 * UNIT], bf16, tag="h2sb",
                                       name=f"h2sb{u}")
                    drain(h2sb, h2ps, "relu", b2_sb)
                    h2_tiles[u] = h2sb

                def stage_l3(u):
                    c = u // UPC
                    if u % UPC == 0:
                        o_tiles[c] = opool.tile([128, CHUNK], bf16, tag="o",
                                                name=f"o_t{c}")
                    o_t = o_tiles[c]
                    h2sb = h2_tiles.pop(u)
                    # merge two units' L3 outputs into one 2-bank PSUM tile
                    # so the out drain is 1024 wide (one op per 2 units)
                    if u % 2 == 0:
                        ops_pair[0] = pso.tile([128, 2 * UNIT], f32,
                                               tag="po", name=f"ops{u}")
                    ops = ops_pair[0]
                    off = (u % 2) * UNIT
                    nc.tensor.matmul(ops[0:OUTP, off:off + NT], w3_sb,
                                     h2sb[:, 0:NT],
                                     start=True, stop=True,
                                     tile_position=(0, 0))
                    nc.tensor.matmul(ops[OUTP:128, off:off + NT], w3_sb,
                                     h2sb[:, NT:2 * NT],
                                     start=True, stop=True,
                                     tile_position=(0, OUTP))
                    if u % 2 == 1:
                        xs = slice((u % UPC - 1) * UNIT,
                                   (u % UPC + 1) * UNIT)
                        drain(o_t[:, xs], ops, "ident", b3_sb)
                    if u % UPC == UPC - 1:
                        nc.sync.dma_start(
                            out=out_d[:, c * CHUNK:(c + 1) * CHUNK],
                            in_=o_t)
                        del o_tiles[c]

                D = SKEWD
                for u in range(NU + 2 * D):
                    if u < NU:
                        stage_l1(u)
                    if D <= u < NU + D:
                        stage_l2(u - D)
                    if 2 * D <= u:
                        stage_l3(u - 2 * D)

            loop = (tc.For_i(0, reps, 1,
                             hint_engines=(mybir.EngineType.PE,),
                             staggered_reset=True)
                    if reps is not None else contextlib.nullcontext())
            with loop:
                body()

    nc.compile()
    return nc


def _get_nc():
    global _CACHED
    if _CACHED is None:
        _CACHED = _build_nc()
    return _CACHED


def _prep_member(x_e, W1_e, b1_e, W2_e, b2_e, W3_e, b3_e):
    """Host-side shard prep: transpose to feature-major, pack the two batch
    halves on the partition axis, clip action features, cast to bf16."""
    xt = np.ascontiguousarray(np.asarray(x_e).T)      # [64, B] f32
    np.clip(xt[IN - AC:IN], -1.0, 1.0, out=xt[IN - AC:IN])
    X = np.empty((128, HB), dtype=BF16)
    X[0:64] = xt[:, :HB]
    X[64:128] = xt[:, HB:]

    w1p = np.empty((128, H), dtype=BF16)
    w1p[0:64] = W1_e
    w1p[64:128] = W1_e
    w2 = W2_e.astype(BF16)
    w3p = np.zeros((H, OUTP), dtype=BF16)
    w3p[:, :OUT] = W3_e
    b1v = np.ascontiguousarray(b1_e.astype(np.float32).reshape(H, 1))
    b2v = np.ascontiguousarray(b2_e.astype(np.float32).reshape(H, 1))
    b3v = np.zeros((128, 1), dtype=np.float32)
    b3v[0:OUT, 0] = b3_e
    b3v[OUTP:OUTP + OUT, 0] = b3_e
    return {"x": X, "w1p": w1p, "w2": w2, "w3p": w3p,
            "b1v": b1v, "b2v": b2v, "b3v": b3v}


def kernel(**inputs):
    from concourse.bass_utils import run_bass_kernel_spmd

    x = np.asarray(inputs["inputs"], dtype=np.float32).reshape(E, B, IN)
    W1 = np.asarray(inputs["W1"], dtype=np.float32)
    b1 = np.asarray(inputs["b1"], dtype=np.float32)
    W2 = np.asarray(inputs["W2"], dtype=np.float32)
    b2 = np.asarray(inputs["b2"], dtype=np.float32)
    W3 = np.asarray(inputs["W3"], dtype=np.float32)
    b3 = np.asarray(inputs["b3"], dtype=np.float32)

    in_maps = [
        _prep_member(x[e], W1[e], b1[e], W2[e], b2[e], W3[e], b3[e])
        for e in range(E)
    ]

    nc = _get_nc()
    res = run_bass_kernel_spmd(nc, in_maps, core_ids=list(range(E)))

    out = np.empty((E, B, OUT), dtype=np.float32)
    for e in range(E):
        dev = res.results[e]["out"]          # [128, HB] bf16
        out[e, :HB] = dev[0:OUT, :].T
        out[e, HB:] = dev[OUTP:OUTP + OUT, :].T
    return out
